# revision 24
# baseline (speedup 1.0000x reference)
"""Trainium2 Bass kernel for the GAWA decoder (2-layer GRU + degenerate
single-position cross-attention + vocab projection), data-parallel over 8
NeuronCores.

Exact algebraic simplifications:
  * softmax over a length-1 axis is 1, so attention collapses:
    logits(t) = (h1(t) + ao) @ proj_w.T + proj_b with per-batch constant
    ao; aop = ao @ proj_w.T + proj_b is precomputed and injected into the
    logits PSUM with an identity matmul.
  * layer-0 input gates split as ce(t) @ w_ce.T + ge where
    ge = eword @ w_e.T + biases is a per-batch constant injected via an
    identity matmul into each gate PSUM accumulation.

All matmuls run fp16 (measured: fp8 DoubleRow on this hardware streams
the moving operand at the same 1 column/cycle as fp16, so fp8 buys no
PE time and only adds cast ops and quantization error).  Gate operands
are pre-scaled by S=256 host-side and the activation's free scale
parameter removes it, which lets all layer-0 biases fold into the ge
constant so its sigmoid/tanh need no bias and can run as single wide
(multi-bank) activations.

Pipelining: layer 0 runs one decode step ahead of layer 1; their
PE/ACT/DVE work is interleaved per iteration so both GRU chains advance
concurrently.  PSUM is cycled in 2-bank quanta: r-phase and z-phase of
each layer reuse one 2-bank tag per layer (A=L0+logits, B=L1), plus a
2-bank n-gate tag per layer (C0/C1) — exactly 8 banks.

Gate math: r is multiplied into the n-gate PSUM in place via
scalar_tensor_tensor ((hn + b_hh_n) * r); the input-side matmuls then
accumulate on top (has_written bits survive the DVE write).

Logits PSUM alternates between the A and B tags each step so the next
iteration's gate matmuls never wait on the logits evacuation, and the
aop constant (+proj_b) is added during the single DVE evacuation op.
"""

import os
import sys

for _p in ("/opt/trn_rl_repo", "/root/.axon_site/_ro/trn_rl_repo"):
    if os.path.isdir(_p) and _p not in sys.path:
        sys.path.insert(0, _p)

import numpy as np
import ml_dtypes

import concourse.bacc as bacc
import concourse.mybir as mybir
import concourse.tile as tile
from concourse.bass_utils import run_bass_kernel_spmd

B, T, V = 4096, 32, 256
E, CE, H = 768, 64, 256
NCORES = 8
BP = B // NCORES  # 512 batch rows per core
BOS, PAD = 1, 0

S_H = 16.0
S_W = 16.0
S = S_H * S_W
INV_S = 1.0 / S

F8 = mybir.dt.float8e4
F16 = mybir.dt.float16
F32 = mybir.dt.float32
AF = mybir.ActivationFunctionType
ALU = mybir.AluOpType
DR = mybir.MatmulPerfMode.DoubleRow

_BC_GE = 0    # 6: (b_ih0 + b_hh0 for r,z) * S per gate row-tile
_BC_H0 = 6    # 2: eword_proj_b (unscaled)
_BC_VAL = 8   # 2: val_b
_BC_BV = 10   # 2: attn_in_b v-part
_BC_AO = 12   # 2: attn_out_b
_BC_HN0 = 14  # 2: b_hh0 n-part * S  (STT scalar, L0)
_BC_HN1 = 16  # 2: b_hh1 n-part * S  (STT scalar, L1)
_BC_RZ1 = 18  # 4: (b_ih1 + b_hh1) r,z (unscaled, sigmoid bias)
_BC_IN1 = 22  # 2: b_ih1 n-part (unscaled, tanh bias)
_NBC = 24

_CACHE = {}


def _build_nc():
    nc = bacc.Bacc("TRN2", target_bir_lowering=False, debug=False,
                   num_devices=NCORES)

    dt = nc.dram_tensor
    ewordT = dt("ewordT", [E, BP], F16, kind="ExternalInput")
    ceT = dt("ceT", [CE, T, BP], F16, kind="ExternalInput")
    weT_s = dt("weT_s", [E, 3 * H], F16, kind="ExternalInput")
    wceT_s = dt("wceT_s", [CE, 3 * H], F16, kind="ExternalInput")
    eprojT = dt("eprojT", [E, H], F16, kind="ExternalInput")
    valT = dt("valT", [E, H], F16, kind="ExternalInput")
    wvT = dt("wvT", [H, H], F16, kind="ExternalInput")
    aowT = dt("aowT", [H, H], F16, kind="ExternalInput")
    projT = dt("projT", [H, V], F16, kind="ExternalInput")
    whh0T_s = dt("whh0T_s", [H, 3 * H], F16, kind="ExternalInput")
    whh1T_s = dt("whh1T_s", [H, 3 * H], F16, kind="ExternalInput")
    wih1T_s = dt("wih1T_s", [H, 3 * H], F16, kind="ExternalInput")
    projb_row = dt("projb_row", [1, V], F16, kind="ExternalInput")
    biasN = dt("biasN", [128, _NBC], F32, kind="ExternalInput")
    ident_d = dt("ident", [128, 128], F16, kind="ExternalInput")
    ones1_d = dt("ones1", [1, 128], F16, kind="ExternalInput")
    out_d = dt("out", [T, 4, 128, V], F16, kind="ExternalOutput")

    with tile.TileContext(nc) as tc:
        with (
            tc.tile_pool(name="wpool", bufs=1) as wp,
            tc.tile_pool(name="cpool", bufs=1) as cp,
            tc.tile_pool(name="h0p", bufs=2) as h0p,
            tc.tile_pool(name="h1p", bufs=2) as h1p,
            tc.tile_pool(name="gp", bufs=2) as gp,
            tc.tile_pool(name="lp", bufs=2) as lp,
            tc.tile_pool(name="psp", bufs=1, space="PSUM") as psp,
        ):
            dma = nc.sync.dma_start
            mm = nc.tensor.matmul
            act = nc.scalar.activation
            stt = nc.vector.scalar_tensor_tensor

            _dma_engines = [nc.sync, nc.scalar, nc.gpsimd]
            _dma_rr = [0]

            def ldma(dst, src_):
                eng = _dma_engines[_dma_rr[0] % len(_dma_engines)]
                _dma_rr[0] += 1
                eng.dma_start(dst, src_)

            def load2d(dram, rows, cols, tag):
                tiles = []
                for k in range(rows // 128):
                    tl = wp.tile([128, cols], F16, tag=f"{tag}{k}")
                    ldma(tl[:], dram[k * 128:(k + 1) * 128, :])
                    tiles.append(tl)
                return tiles

            # DMA priority order: the ge matmuls need (ew[k], we[k])
            # pairwise; the first L0 cell needs bias, wce, ce[:,0:8], whh0
            # and h0-init needs eproj.  Attention/logits/L1 weights follow.
            ew, we = [], []
            for k in range(6):
                tl = wp.tile([128, BP], F16, tag=f"ew{k}", name=f"ew{k}")
                ldma(tl[:], ewordT[k * 128:(k + 1) * 128, :])
                ew.append(tl)
                tl2 = wp.tile([128, 3 * H], F16, tag=f"we{k}", name=f"we{k}")
                ldma(tl2[:], weT_s[k * 128:(k + 1) * 128, :])
                we.append(tl2)
            bias = wp.tile([128, _NBC], F32, tag="bias")
            ldma(bias[:], biasN[:])
            wce = wp.tile([CE, 3 * H], F16, tag="wce")
            ldma(wce[:], wceT_s[:])
            ident = wp.tile([128, 128], F16, tag="ident")
            ldma(ident[:], ident_d[:])
            eproj = load2d(eprojT, E, H, "eproj")
            ce_sb = cp.tile([CE, T, BP], F16, tag="ce")
            ldma(ce_sb[:, 0:8, :], ceT[:, 0:8, :])
            whh0 = load2d(whh0T_s, H, 3 * H, "whh0")
            whh1 = load2d(whh1T_s, H, 3 * H, "whh1")
            wih1 = load2d(wih1T_s, H, 3 * H, "wih1")
            val = load2d(valT, E, H, "val")
            wv_t = load2d(wvT, H, H, "wv")
            aow = load2d(aowT, H, H, "aow")
            proj = load2d(projT, H, V, "proj")
            pbrow = wp.tile([1, V], F16, tag="pbrow")
            ldma(pbrow[:], projb_row[:])
            ones1 = wp.tile([1, 128], F16, tag="ones1")
            ldma(ones1[:], ones1_d[:])
            for q in range(1, 4):
                ldma(ce_sb[:, q * 8:(q + 1) * 8, :],
                     ceT[:, q * 8:(q + 1) * 8, :])

            def bcol(c):
                return bias[:, c:c + 1]

            def psA():
                ps = psp.tile([128, 2, BP], F32, tag="A", name="psA")
                return ps

            def psB():
                ps = psp.tile([128, 2, BP], F32, tag="B", name="psB")
                return ps

            def psC0():
                ps = psp.tile([128, 2, BP], F32, tag="C0", name="psC0")
                return ps

            def psC1():
                ps = psp.tile([128, 2, BP], F32, tag="C1", name="psC1")
                return ps

            def psLG(tag="A"):
                ps = psp.tile([128, 4, V], F32, tag=tag, name="psLG")
                return ps

            # ---------------- prologue ----------------
            # ge[g] = (we_s @ ew) + bias*S for 6 gate row-tiles; the n-gate
            # pair (4,5) lands in one contiguous tile for a merged DVE add
            ge = []
            gen = cp.tile([128, 2, BP], F16, tag="gen")
            for half in range(3):
                ps = psA()
                for g2 in range(2):
                    g = half * 2 + g2
                    for k in range(6):
                        mm(ps[:, g2, :], we[k][:, g * 128:(g + 1) * 128],
                           ew[k][:], start=(k == 0), stop=(k == 5))
                for g2 in range(2):
                    g = half * 2 + g2
                    if g < 4:
                        t_ = cp.tile([128, BP], F16, tag=f"ge{g}",
                                     name=f"ge{g}")
                        act(t_[:], ps[:, g2, :], AF.Identity,
                            bias=bcol(_BC_GE + g))
                        ge.append(t_)
                    else:
                        act(gen[:, g - 4, :], ps[:, g2, :], AF.Identity,
                            bias=bcol(_BC_GE + g))
            # h0 init = tanh(eword @ eword_proj_w.T + b)
            psI = psB()
            for m_ in range(2):
                for k in range(6):
                    mm(psI[:, m_, :], eproj[k][:, m_ * 128:(m_ + 1) * 128],
                       ew[k][:], start=(k == 0), stop=(k == 5))
            h0f = h0p.tile([128, 2, BP], F16, tag="h0f")
            h1f = h1p.tile([128, 2, BP], F16, tag="h1f")
            for m_ in range(2):
                act(h0f[:, m_, :], psI[:, m_, :], AF.Tanh,
                    bias=bcol(_BC_H0 + m_))
            nc.vector.tensor_copy(h1f[:], h0f[:])

            # ao = ((ew@val.T+vb)@wv.T+bv)@aow.T+aob, then aop = ao@proj+pb
            psC = psC0()
            for m_ in range(2):
                for k in range(6):
                    mm(psC[:, m_, :], val[k][:, m_ * 128:(m_ + 1) * 128],
                       ew[k][:], start=(k == 0), stop=(k == 5))
            v1s = gp.tile([128, 2, BP], F16, tag="v1s")
            for m_ in range(2):
                act(v1s[:, m_, :], psC[:, m_, :], AF.Identity,
                    bias=bcol(_BC_VAL + m_))
            psD = psC1()
            for m_ in range(2):
                for k in range(2):
                    mm(psD[:, m_, :], wv_t[k][:, m_ * 128:(m_ + 1) * 128],
                       v1s[:, k, :], start=(k == 0), stop=(k == 1))
            evs = gp.tile([128, 2, BP], F16, tag="v1s")
            for m_ in range(2):
                act(evs[:, m_, :], psD[:, m_, :], AF.Identity,
                    bias=bcol(_BC_BV + m_))
            psE = psC0()
            for m_ in range(2):
                for k in range(2):
                    mm(psE[:, m_, :], aow[k][:, m_ * 128:(m_ + 1) * 128],
                       evs[:, k, :], start=(k == 0), stop=(k == 1))
            aos = gp.tile([128, 2, BP], F16, tag="v1s")
            for m_ in range(2):
                act(aos[:, m_, :], psE[:, m_, :], AF.Identity,
                    bias=bcol(_BC_AO + m_))
            psF = psLG()
            for mb in range(4):
                for k in range(2):
                    mm(psF[:, mb, :],
                       aos[:, k, mb * 128:(mb + 1) * 128], proj[k][:],
                       start=(k == 0), stop=False)
                mm(psF[:, mb, :], ones1[:], pbrow[:], start=False, stop=True)
            aop = cp.tile([128, 4, V], F16, tag="aop")
            act(aop[:], psF[:], AF.Copy)

            # ---------------- emission helpers ----------------
            def l0_gate_mms(tin, hf, gates):
                ps = psA()
                ce_t = ce_sb[:, tin, :]
                for g2, g in enumerate(gates):
                    mm(ps[:, g2, :], ident[:], ge[g][:], start=True, stop=False)
                    mm(ps[:, g2, :], wce[:, g * 128:(g + 1) * 128], ce_t,
                       start=False, stop=False)
                    for k in range(2):
                        mm(ps[:, g2, :], whh0[k][:, g * 128:(g + 1) * 128],
                           hf[:, k, :], start=False, stop=(k == 1))
                return ps

            def l0_r_mms(tin, hf):
                return l0_gate_mms(tin, hf, (0, 1))

            def l0_z_mms(tin, hf):
                return l0_gate_mms(tin, hf, (2, 3))

            def l0_hn_mms(hf):
                ps = psC0()
                for i in range(2):
                    for k in range(2):
                        mm(ps[:, i, :],
                           whh0[k][:, 512 + i * 128:512 + (i + 1) * 128],
                           hf[:, k, :], start=(k == 0), stop=False)
                return ps

            def l0_inn_mms(ps, tin):
                ce_t = ce_sb[:, tin, :]
                for i in range(2):
                    mm(ps[:, i, :], wce[:, (4 + i) * 128:(5 + i) * 128],
                       ce_t, start=False, stop=True, skip_group_check=True)

            def l1_rz_mms(gates, h1f_, h0f_):
                ps = psB()
                for g2, g in enumerate(gates):
                    for k in range(2):
                        mm(ps[:, g2, :], whh1[k][:, g * 128:(g + 1) * 128],
                           h1f_[:, k, :], start=(k == 0), stop=False)
                    for k in range(2):
                        mm(ps[:, g2, :], wih1[k][:, g * 128:(g + 1) * 128],
                           h0f_[:, k, :], start=False, stop=(k == 1))
                return ps

            def l1_hn_mms(h1f_):
                ps = psC1()
                for i in range(2):
                    for k in range(2):
                        mm(ps[:, i, :],
                           whh1[k][:, 512 + i * 128:512 + (i + 1) * 128],
                           h1f_[:, k, :], start=(k == 0), stop=False)
                return ps

            def l1_inn_mms(ps, h0f_):
                for i in range(2):
                    for k in range(2):
                        mm(ps[:, i, :],
                           wih1[k][:, 512 + i * 128:512 + (i + 1) * 128],
                           h0f_[:, k, :], start=False, stop=(k == 1),
                           skip_group_check=True)

            def emit_logits_mms(hf, tag):
                # aop + proj_b are added during the DVE evacuation
                ps = psLG(tag)
                for mb in range(4):
                    for k in range(2):
                        mm(ps[:, mb, :],
                           hf[:, k, mb * 128:(mb + 1) * 128], proj[k][:],
                           start=(k == 0), stop=(k == 1))
                return ps

            # ---------------- L0 step 0 (prologue cell) ----------------
            Ar = l0_r_mms(0, h0f)
            r0s = gp.tile([128, 2, BP], F16, tag="r0")
            act(r0s[:], Ar[:], AF.Sigmoid, scale=INV_S)
            C0 = l0_hn_mms(h0f)
            for i in range(2):
                stt(C0[:, i, :], C0[:, i, :], bcol(_BC_HN0 + i),
                    r0s[:, i, :], ALU.add, ALU.mult)
            nc.vector.tensor_add(C0[:], C0[:], gen[:])
            Az = l0_z_mms(0, h0f)
            l0_inn_mms(C0, 0)
            z0s = gp.tile([128, 2, BP], F16, tag="z0")
            act(z0s[:], Az[:], AF.Sigmoid, scale=INV_S)
            n0s = gp.tile([128, 2, BP], F16, tag="n0")
            act(n0s[:], C0[:], AF.Tanh, scale=INV_S)
            d0 = gp.tile([128, 2, BP], F16, tag="d0")
            nc.vector.tensor_sub(d0[:], h0f[:], n0s[:])
            e0 = gp.tile([128, 2, BP], F16, tag="e0")
            nc.vector.tensor_mul(e0[:], d0[:], z0s[:])
            h0prev = h0f
            h0f = h0p.tile([128, 2, BP], F16, tag="h0f")
            nc.vector.tensor_add(h0f[:], e0[:], n0s[:])

            # ---------------- the scan ----------------
            for t in range(T):
                do_l0 = t < T - 1
                h0_in = h0f  # h0 state (t)
                h1_in = h1f  # h1 state (t-1)

                # --- r-phase matmuls; L1 first so the previous logits
                # evacuation (A-tag) drains under the B/C1 matmuls ---
                Br = l1_rz_mms((0, 1), h1f, h0f)
                C1 = l1_hn_mms(h1f)
                if do_l0:
                    Ar = l0_r_mms(t + 1, h0f)
                    C0 = l0_hn_mms(h0f)

                # --- sigmoid r ---
                if do_l0:
                    r0s = gp.tile([128, 2, BP], F16, tag="r0")
                    act(r0s[:], Ar[:], AF.Sigmoid, scale=INV_S)
                r1s = gp.tile([128, 2, BP], F16, tag="r1")
                for i in range(2):
                    act(r1s[:, i, :], Br[:, i, :], AF.Sigmoid,
                        bias=bcol(_BC_RZ1 + i), scale=INV_S)

                # --- (hn + b) * r in place ---
                if do_l0:
                    for i in range(2):
                        stt(C0[:, i, :], C0[:, i, :], bcol(_BC_HN0 + i),
                            r0s[:, i, :], ALU.add, ALU.mult)
                    nc.vector.tensor_add(C0[:], C0[:], gen[:])
                for i in range(2):
                    stt(C1[:, i, :], C1[:, i, :], bcol(_BC_HN1 + i),
                        r1s[:, i, :], ALU.add, ALU.mult)

                # --- z-phase matmuls + input-side n accumulation ---
                if do_l0:
                    Az = l0_z_mms(t + 1, h0f)
                    l0_inn_mms(C0, t + 1)
                Bz = l1_rz_mms((2, 3), h1f, h0f)
                l1_inn_mms(C1, h0f)

                # --- sigmoid z, tanh (L0 chain first) ---
                if do_l0:
                    z0s = gp.tile([128, 2, BP], F16, tag="z0")
                    act(z0s[:], Az[:], AF.Sigmoid, scale=INV_S)
                    n0s = gp.tile([128, 2, BP], F16, tag="n0")
                    act(n0s[:], C0[:], AF.Tanh, scale=INV_S)
                z1s = gp.tile([128, 2, BP], F16, tag="z1")
                for i in range(2):
                    act(z1s[:, i, :], Bz[:, i, :], AF.Sigmoid,
                        bias=bcol(_BC_RZ1 + 2 + i), scale=INV_S)
                n1s = gp.tile([128, 2, BP], F16, tag="n1")
                for i in range(2):
                    act(n1s[:, i, :], C1[:, i, :], AF.Tanh,
                        bias=bcol(_BC_IN1 + i), scale=INV_S)

                # --- h updates: h' = z*(h - n) + n ---
                if do_l0:
                    d0 = gp.tile([128, 2, BP], F16, tag="d0")
                    nc.vector.tensor_sub(d0[:], h0_in[:], n0s[:])
                    e0 = gp.tile([128, 2, BP], F16, tag="e0")
                    nc.vector.tensor_mul(e0[:], d0[:], z0s[:])
                    h0f = h0p.tile([128, 2, BP], F16, tag="h0f")
                    nc.vector.tensor_add(h0f[:], e0[:], n0s[:])
                d1 = gp.tile([128, 2, BP], F16, tag="d1")
                nc.vector.tensor_sub(d1[:], h1_in[:], n1s[:])
                e1 = gp.tile([128, 2, BP], F16, tag="e1")
                nc.vector.tensor_mul(e1[:], d1[:], z1s[:])
                h1f = h1p.tile([128, 2, BP], F16, tag="h1f")
                nc.vector.tensor_add(h1f[:], e1[:], n1s[:])

                # --- logits(t-1); PSUM tag alternates A/B so the next
                # iteration's gate matmuls never wait on this evacuation ---
                if t > 0:
                    lg = emit_logits_mms(h1_in, "A" if t % 2 == 0 else "B")
                    lo = lp.tile([128, 4, V], F16, tag="lo")
                    nc.vector.tensor_add(lo[:], lg[:], aop[:])
                    for mb in range(4):
                        dma(out_d[t - 1, mb], lo[:, mb, :])

            # epilogue: logits for step T-1
            lg = emit_logits_mms(h1f, "A")
            lo = lp.tile([128, 4, V], F16, tag="lo")
            nc.vector.tensor_add(lo[:], lg[:], aop[:])
            for mb in range(4):
                dma(out_d[T - 1, mb], lo[:, mb, :])

    nc.compile()
    return nc


def _q8(x, scale):
    return np.clip(x * scale, -240.0, 240.0).astype(ml_dtypes.float8_e4m3)


def kernel(**inputs):
    eword = np.ascontiguousarray(inputs["eword"], dtype=np.float32)
    target_ids = np.asarray(inputs["target_ids"])
    char_emb = np.asarray(inputs["char_emb"], dtype=np.float32)
    w_ih0 = np.asarray(inputs["gru_w_ih0"], dtype=np.float32)
    w_hh0 = np.asarray(inputs["gru_w_hh0"], dtype=np.float32)
    b_ih0 = np.asarray(inputs["gru_b_ih0"], dtype=np.float32)
    b_hh0 = np.asarray(inputs["gru_b_hh0"], dtype=np.float32)
    w_ih1 = np.asarray(inputs["gru_w_ih1"], dtype=np.float32)
    w_hh1 = np.asarray(inputs["gru_w_hh1"], dtype=np.float32)
    b_ih1 = np.asarray(inputs["gru_b_ih1"], dtype=np.float32)
    b_hh1 = np.asarray(inputs["gru_b_hh1"], dtype=np.float32)
    attn_in_w = np.asarray(inputs["attn_in_w"], dtype=np.float32)
    attn_in_b = np.asarray(inputs["attn_in_b"], dtype=np.float32)
    attn_out_w = np.asarray(inputs["attn_out_w"], dtype=np.float32)
    attn_out_b = np.asarray(inputs["attn_out_b"], dtype=np.float32)
    eword_proj_w = np.asarray(inputs["eword_proj_w"], dtype=np.float32)
    eword_proj_b = np.asarray(inputs["eword_proj_b"], dtype=np.float32)
    val_w = np.asarray(inputs["val_w"], dtype=np.float32)
    val_b = np.asarray(inputs["val_b"], dtype=np.float32)
    proj_w = np.asarray(inputs["proj_w"], dtype=np.float32)
    proj_b = np.asarray(inputs["proj_b"], dtype=np.float32)

    f16 = np.float16

    in_ids = np.concatenate(
        [np.full((B, 1), BOS, target_ids.dtype), target_ids[:, :-1]], axis=1)
    ce = char_emb[in_ids] * (in_ids != PAD)[..., None].astype(np.float32)

    wv = attn_in_w[2 * H:3 * H]
    bv = attn_in_b[2 * H:3 * H]

    shared = {
        "weT_s": np.ascontiguousarray(w_ih0[:, CE:].T * S, dtype=f16),
        "wceT_s": np.ascontiguousarray(w_ih0[:, :CE].T * S, dtype=f16),
        "eprojT": np.ascontiguousarray(eword_proj_w.T, dtype=f16),
        "valT": np.ascontiguousarray(val_w.T, dtype=f16),
        "wvT": np.ascontiguousarray(wv.T, dtype=f16),
        "aowT": np.ascontiguousarray(attn_out_w.T, dtype=f16),
        "projT": np.ascontiguousarray(proj_w.T, dtype=f16),
        "whh0T_s": np.ascontiguousarray(w_hh0.T * S, dtype=f16),
        "whh1T_s": np.ascontiguousarray(w_hh1.T * S, dtype=f16),
        "wih1T_s": np.ascontiguousarray(w_ih1.T * S, dtype=f16),
        "projb_row": np.ascontiguousarray(proj_b[None, :], dtype=f16),
        "ident": np.eye(128, dtype=f16),
        "ones1": np.ones((1, 128), dtype=f16),
    }

    bias = np.zeros((128, _NBC), np.float32)
    for g in range(6):
        col = b_ih0[g * 128:(g + 1) * 128].copy()
        if g < 4:
            col += b_hh0[g * 128:(g + 1) * 128]
        bias[:, _BC_GE + g] = col * S
    b1 = b_ih1 + b_hh1
    for m in range(2):
        bias[:, _BC_H0 + m] = eword_proj_b[m * 128:(m + 1) * 128]
        bias[:, _BC_VAL + m] = val_b[m * 128:(m + 1) * 128]
        bias[:, _BC_BV + m] = bv[m * 128:(m + 1) * 128]
        bias[:, _BC_AO + m] = attn_out_b[m * 128:(m + 1) * 128]
        bias[:, _BC_HN0 + m] = b_hh0[2 * H + m * 128:2 * H + (m + 1) * 128] * S
        bias[:, _BC_HN1 + m] = b_hh1[2 * H + m * 128:2 * H + (m + 1) * 128] * S
        bias[:, _BC_IN1 + m] = b_ih1[2 * H + m * 128:2 * H + (m + 1) * 128]
    for g in range(4):
        bias[:, _BC_RZ1 + g] = b1[g * 128:(g + 1) * 128]
    shared["biasN"] = bias

    in_maps = []
    for c in range(NCORES):
        sl = slice(c * BP, (c + 1) * BP)
        m = dict(shared)
        m["ewordT"] = np.ascontiguousarray(eword[sl].T, dtype=f16)
        m["ceT"] = np.ascontiguousarray(
            ce[sl].transpose(2, 1, 0), dtype=f16)  # (CE, T, BP)
        in_maps.append(m)

    if "nc" not in _CACHE:
        _CACHE["nc"] = _build_nc()
    nc = _CACHE["nc"]

    res = run_bass_kernel_spmd(nc, in_maps, list(range(NCORES)),
                               trace=bool(os.environ.get("BASS_TRACE")))
    _CACHE["last_res"] = res
    outs = []
    for c in range(NCORES):
        o = res.results[c]["out"]  # (T, 4, 128, V) f16
        outs.append(np.asarray(o).reshape(T, BP, V).transpose(1, 0, 2))
    return np.ascontiguousarray(np.concatenate(outs, axis=0),
                                dtype=np.float32)
